# revision 16
# baseline (speedup 1.0000x reference)
"""Trainium2 Bass kernel: Bahdanau-style attention
    out = softmax_S( V . tanh(enc @ W1^T + h @ W2^T + b1 + b2) )
Data-parallel over batch across 8 NeuronCores; weights replicated.

v2: mixed-precision mains + per-batch softmax tail.

Mains (the 512 contraction): chunks h<256 go through ONE fp8e4 DoubleRow
matmul (2 k-subtiles, 2 MACs/cycle); chunks h>=256 stay bf16 (2 MMs).
1602 PE cycles per (oc, half) vs 2048 all-bf16. Host pre-scales
enc8 x16 / W1_8 x256 / W1_bf x4096 so all PSUM contributions share one
2^12 scale, undone by the tanh activation's scale=2^-12. Accuracy gate:
sim 1.65e-2 vs the 2e-2 budget (all-fp8 sims at 2.13e-2 -> fails; e3m4
would pass at 9.4e-3 but the BIR verifier rejects e3 DoubleRow).

V-dot: per (b, pair, half) 4 col-tiled concurrent matvecs put V.energy
partials on partitions {0,32,64,96} of a persistent (memset-once) PSUM
bank; DVE copies to SBUF; a 0/1-mask matvec combines them and lands the
unit's [1,512] scores at partition 32*(2p+half) of a per-batch collect
bank (tile_position col offset). When a batch's 4 units are in, ONE
ScalarE exp [128,512] (+accum per-partition dens) replaces the old
per-pair [1,1024] single-lane exps; a ones-matrix f32 matvec replicates
sum-of-4-dens to all partitions; DVE reciprocal + tensor_scalar_mul
normalize; one strided DMA writes rows {0,32,64,96} as out[b, 2048].

Two-deep software pipeline as before: iter k runs mains(k) |
colmv(k-1)+copy | mask/exp(k-2) so the PE never waits on tanh or DVE.
"""

import sys
import types

if "/opt/trn_rl_repo" not in sys.path:
    sys.path.insert(0, "/opt/trn_rl_repo")

import numpy as np
import ml_dtypes

N_CORES = 8
B, S, H = 64, 2048, 512
BPC = B // N_CORES          # batches per core
NCH = H // 128              # 4 partition-chunks of the hidden dim
SBLK = 512                  # one PSUM bank of f32
PW = 2 * SBLK               # pair width
NPAIR = S // PW             # 2 pairs per batch

ENC8_SCALE = 16.0           # enc fp8 pre-scale (host)
W18_SCALE = 256.0           # W1 fp8 rows pre-scale (host)
WBF_SCALE = ENC8_SCALE * W18_SCALE   # bf16 W1 rows pre-scale (host)
ACT_SCALE = 1.0 / WBF_SCALE          # undo in the tanh activation

TRACE = False               # test.py flips this to profile
LAST_EXEC_NS = None
LAST_RESULT = None

_cache = {}


def _install_profile_hook():
    """Best-effort: register the NTFF profile hook that this container's
    boot skips because antenv.axon_hooks is absent."""
    try:
        import antenv
        if getattr(antenv, "axon_hooks", None) is not None:
            return
        import trn_agent_boot.trn_boot as tb
        hooks = types.ModuleType("antenv.axon_hooks")
        _h = [None]
        hooks.set_axon_ntff_profile_hook = lambda h: _h.__setitem__(0, h)
        hooks.get_axon_ntff_profile_hook = lambda: _h[0]
        sys.modules["antenv.axon_hooks"] = hooks
        antenv.axon_hooks = hooks
        hooks.set_axon_ntff_profile_hook(
            tb._ntff_profile_via_ctypes("/opt/axon/libaxon_pjrt.so"))
        import concourse.bass_utils as bu
        bu.upload_artifacts = lambda d: "local://" + d
    except Exception:
        pass


def _build_nc():
    import concourse.tile as tile
    from concourse import bacc, mybir

    f32 = mybir.dt.float32
    bf16 = mybir.dt.bfloat16
    fp8 = mybir.dt.float8e4
    AF = mybir.ActivationFunctionType
    DR = mybir.MatmulPerfMode.DoubleRow

    nc = bacc.Bacc("TRN2", target_bir_lowering=False, debug=False,
                   num_devices=N_CORES)

    # h<256 rows of encT/W1T in fp8 (DoubleRow), h>=256 rows in bf16
    enc8 = nc.dram_tensor("enc8", [BPC, 2 * 128, S], fp8,
                          kind="ExternalInput").ap()
    encb = nc.dram_tensor("encb", [BPC, 2 * 128, S], bf16,
                          kind="ExternalInput").ap()
    w1t8 = nc.dram_tensor("w1t8", [2 * 128, H], fp8,
                          kind="ExternalInput").ap()
    w1tb = nc.dram_tensor("w1tb", [2 * 128, H], bf16,
                          kind="ExternalInput").ap()
    hT = nc.dram_tensor("hT", [H, BPC], bf16, kind="ExternalInput").ap()
    w2t = nc.dram_tensor("w2t", [H, H], bf16, kind="ExternalInput").ap()
    vre = nc.dram_tensor("vre", [128, NCH + 1], bf16,
                         kind="ExternalInput").ap()
    bre = nc.dram_tensor("bre", [128, NCH], f32, kind="ExternalInput").ap()
    ones = nc.dram_tensor("ones", [128, 128], f32,
                          kind="ExternalInput").ap()
    out = nc.dram_tensor("out", [BPC, S], f32, kind="ExternalOutput").ap()

    with tile.TileContext(nc) as tc:
        with (
            tc.tile_pool(name="consts", bufs=1) as consts,
            tc.tile_pool(name="enc", bufs=4) as encp,
            tc.tile_pool(name="energy", bufs=3) as energyp,
            tc.tile_pool(name="partsb", bufs=4) as partsbp,
            tc.tile_pool(name="expp", bufs=2) as expp,
            tc.tile_pool(name="psum_proj", bufs=2, space="PSUM") as projp,
            tc.tile_pool(name="psum_part", bufs=1, space="PSUM") as partp,
            tc.tile_pool(name="psum_coll", bufs=1, space="PSUM") as collp,
        ):
            w1t8_sb = consts.tile([128, 2, H], fp8)
            w1tb_sb = consts.tile([128, 2, H], bf16)
            w2t_sb = consts.tile([128, NCH, H], bf16)
            hT_sb = consts.tile([128, NCH, BPC], bf16)
            vre_sb = consts.tile([128, NCH + 1], bf16)
            bre_sb = consts.tile([128, NCH], f32)
            ones_sb = consts.tile([128, 128], f32)
            cbias_sb = consts.tile([128, NCH, BPC], f32)

            # Startup DMA priority, split across the two HWDGE queues so
            # descriptor issue overlaps: sync takes the small cbias inputs
            # (w2t/hT/bre — the cbias matmuls run during the first enc wait
            # instead of stalling the PE mid-stream) and the p>0 enc stream;
            # the idle ACT queue takes W1 and the first enc pair. vre/ones
            # only gate the (lagged) first V-dot and softmax.
            nc.sync.dma_start(hT_sb[:, :, :],
                              hT.rearrange("(c q) o -> q c o", c=NCH))
            nc.sync.dma_start(bre_sb[:, :], bre[:, :])
            nc.sync.dma_start(w2t_sb[:, :, :],
                              w2t.rearrange("(c q) o -> q c o", c=NCH))
            nc.sync.dma_start(w1t8_sb[:, :, :],
                             w1t8.rearrange("(c q) o -> q c o", c=2))
            nc.sync.dma_start(w1tb_sb[:, :, :],
                             w1tb.rearrange("(c q) o -> q c o", c=2))

            def emit_weights2():
                nc.sync.dma_start(vre_sb[:, :], vre[:, :])
                nc.sync.dma_start(ones_sb[:, :], ones[:, :])

            # persistent V-matvec partial banks (alternating per half) +
            # per-batch collect banks: memset ONCE; quadrant/col-offset
            # matmuls only ever write their own partitions and any finite
            # garbage elsewhere is killed by the 0-rows of the mask matvec.
            # part_ps[0] doubles as scratch PSUM for the cbias matmuls and
            # the per-batch den-replication matvec (same garbage argument).
            # The warm-up block is emitted first so its DVE memset (the only
            # thing gating the warm-up matmuls) is at the head of the DVE
            # queue; the PSUM memsets follow (GpSimd has no PSUM port).
            warm_sb = consts.tile([128, SBLK], bf16, name="warm_sb")
            nc.vector.memset(warm_sb[:, :], 0.0)
            warm_ps = projp.tile([128, PW], f32, tag="proj", name="warm_ps")
            for _ in range(8):
                nc.tensor.matmul(warm_ps[:, 0:SBLK], warm_sb[:, 0:128],
                                 warm_sb[:, :], start=True, stop=True)

            part_ps = [partp.tile([128, SBLK], f32, name=f"part{i}")
                       for i in range(2)]
            for t in part_ps:
                nc.vector.memset(t[:, :], 0.0)
            coll_ps = [collp.tile([128, SBLK], f32, name=f"coll{i}")
                       for i in range(2)]
            for t in coll_ps:
                nc.vector.memset(t[:, :], 0.0)

            # cbiasT[o, b] = sum_hin W2T[hin, o] * hT[hin, b] + bsum[o]
            # runs before the first mains, during the enc(b0,p0) DMA window
            # (doubles as HAM warm-up).
            def emit_cbias():
                for oc in range(NCH):
                    pcb = part_ps[0][:, 0:BPC]
                    for hc in range(NCH):
                        nc.tensor.matmul(
                            pcb,
                            w2t_sb[:, hc, oc * 128:(oc + 1) * 128],
                            hT_sb[:, hc, :],
                            start=(hc == 0), stop=(hc == NCH - 1))
                    nc.vector.tensor_scalar_add(
                        cbias_sb[:, oc, :], pcb, bre_sb[:, oc:oc + 1])

            emit_cbias()

            # softmax tail for batch b once its 4 units are in the collect
            # bank: exp+accum, ones-matvec den replication, reciprocal,
            # normalize, strided DMA out.
            def emit_softmax(pb):
                coll = coll_ps[pb % 2]
                exp_sb = expp.tile([128, SBLK], f32, tag="exp")
                den128 = expp.tile([128, 1], f32, tag="den128")
                nc.scalar.activation(exp_sb[:, :], coll[:, :], AF.Exp,
                                     accum_out=den128[:, :])
                den_all = part_ps[0][:, 0:1]
                nc.tensor.matmul(den_all, ones_sb[:, :],
                                 den128[:, :], start=True, stop=True)
                rden = expp.tile([128, 1], f32, tag="rden")
                nc.vector.reciprocal(rden[:, :], den_all)
                norm = expp.tile([128, SBLK], f32, tag="norm")
                nc.vector.tensor_scalar_mul(norm[:, :], exp_sb[:, :],
                                            rden[:, 0:1])
                nc.sync.dma_start(
                    out[pb, :].rearrange("(u s) -> u s", u=4),
                    norm.rearrange("(u q) s -> u q s", u=4)[:, 0, :])

            # two-deep software pipeline behind the main MMs:
            #   iter k: mains(k) | colmv(k-1)+DVE copy | mask(+exp)(k-2)
            pend_colmv = None   # (energy, b, p)
            pend_mask = None    # (psbs, b, p)

            def do_colmv(st):
                energy, pb, pp = st
                psbs = []
                for half in range(2):
                    # 4 concurrent col-tiled matvecs: partial scores land on
                    # partitions {0,32,64,96} of the half's persistent bank
                    pp_ps = part_ps[half]
                    for oc in range(NCH):
                        nc.tensor.matmul(
                            pp_ps[32 * oc:32 * oc + 1, :],
                            vre_sb[:, oc:oc + 1],
                            energy[:, oc, half * SBLK:(half + 1) * SBLK],
                            start=True, stop=True,
                            tile_position=(0, 32 * oc))
                    psb = partsbp.tile([128, SBLK], bf16, tag="partsb")
                    nc.vector.tensor_copy(psb[:, :], pp_ps[:, :])
                    psbs.append(psb)
                return (psbs, pb, pp)

            def do_mask(st):
                psbs, pb, pp = st
                coll = coll_ps[pb % 2]
                for half in range(2):
                    u = 2 * pp + half
                    # combine rows {0,32,64,96} via the 0/1-mask column;
                    # land the unit at partition 32*u of the collect bank
                    nc.tensor.matmul(
                        coll[32 * u:32 * u + 1, :],
                        vre_sb[:, NCH:NCH + 1],
                        psbs[half][:, :],
                        start=True, stop=True,
                        tile_position=(0, 32 * u))
                if pp == NPAIR - 1:
                    emit_softmax(pb)

            for b in range(BPC):
                for p in range(NPAIR):
                    enc8t = encp.tile([128, 2, PW], fp8, tag="enc8")
                    encbt = encp.tile([128, 2, PW], bf16, tag="encb")
                    # the first pair rides the ACT queue (behind W1) so its
                    # issue overlaps the sync queue's weights
                    dq = nc.sync
                    dq.dma_start(
                        enc8t[:, :, :],
                        enc8[b, :, p * PW:(p + 1) * PW]
                        .rearrange("(c q) s -> q c s", c=2))
                    dq.dma_start(
                        encbt[:, :, :],
                        encb[b, :, p * PW:(p + 1) * PW]
                        .rearrange("(c q) s -> q c s", c=2))
                    if b == 0 and p == 0:
                        emit_weights2()
                    energy = energyp.tile([128, NCH, PW], bf16, tag="energy")
                    for oc in range(NCH):
                        ps2 = projp.tile([128, PW], f32, tag="proj")
                        for half in range(2):
                            hs = slice(half * SBLK, (half + 1) * SBLK)
                            nc.tensor.matmul(
                                ps2[:, hs],
                                w1t8_sb[:, :, oc * 128:(oc + 1) * 128],
                                enc8t[:, :, hs],
                                start=True, stop=False, perf_mode=DR)
                            for c in range(2):
                                nc.tensor.matmul(
                                    ps2[:, hs],
                                    w1tb_sb[:, c, oc * 128:(oc + 1) * 128],
                                    encbt[:, c, hs],
                                    start=False, stop=(c == 1))
                        nc.scalar.activation(
                            energy[:, oc, :], ps2[:, :], AF.Tanh,
                            bias=cbias_sb[:, oc, b:b + 1], scale=ACT_SCALE)
                    if pend_colmv is not None:
                        nxt = do_colmv(pend_colmv)
                    else:
                        nxt = None
                    if pend_mask is not None:
                        do_mask(pend_mask)
                    pend_mask = nxt
                    pend_colmv = (energy, b, p)

            # flush: the pending mask's inputs are already in SBUF — emit it
            # first so it fills the PE idle while the last tanh runs.
            if pend_mask is not None:
                do_mask(pend_mask)
            do_mask(do_colmv(pend_colmv))

    nc.compile()
    return nc


def kernel(**inputs):
    global LAST_EXEC_NS, LAST_RESULT
    _install_profile_hook()
    from concourse.bass_utils import run_bass_kernel_spmd

    if "nc" not in _cache:
        _cache["nc"] = _build_nc()
    nc = _cache["nc"]

    h = np.asarray(inputs["h"], dtype=np.float32)            # [1, B, H]
    enc = np.asarray(inputs["enc_out"], dtype=np.float32)    # [B, S, H]
    W1_w = np.asarray(inputs["W1_w"], dtype=np.float32)
    W1_b = np.asarray(inputs["W1_b"], dtype=np.float32)
    W2_w = np.asarray(inputs["W2_w"], dtype=np.float32)
    W2_b = np.asarray(inputs["W2_b"], dtype=np.float32)
    V_w = np.asarray(inputs["V_w"], dtype=np.float32)        # [1, H]

    bf = ml_dtypes.bfloat16
    f8 = ml_dtypes.float8_e4m3
    W1T = W1_w.T                                             # [H(h), H(o)]
    W1T8 = np.ascontiguousarray((W1T[:256] * W18_SCALE).astype(f8))
    W1Tb = np.ascontiguousarray((W1T[256:] * WBF_SCALE).astype(bf))
    W2T = np.ascontiguousarray(W2_w.T.astype(bf))
    vre = np.zeros((128, NCH + 1), dtype=bf)
    vre[:, :NCH] = V_w[0].reshape(NCH, 128).T.astype(bf)
    vre[0::32, NCH] = 1.0
    bre = np.ascontiguousarray((W1_b + W2_b).reshape(NCH, 128).T
                               .astype(np.float32))
    ones = np.zeros((128, 128), dtype=np.float32)
    ones[0::32, :] = 1.0

    in_maps = []
    for c in range(N_CORES):
        sl = slice(c * BPC, (c + 1) * BPC)
        encT = enc[sl].transpose(0, 2, 1)                    # [BPC, H, S]
        enc8 = np.ascontiguousarray(
            (encT[:, :256] * ENC8_SCALE).astype(f8))
        encb = np.ascontiguousarray(encT[:, 256:].astype(bf))
        hTc = np.ascontiguousarray(h[0, sl, :].T.astype(bf)) # [H, BPC]
        in_maps.append({"enc8": enc8, "encb": encb, "w1t8": W1T8,
                        "w1tb": W1Tb, "hT": hTc, "w2t": W2T,
                        "vre": vre, "bre": bre, "ones": ones})

    res = run_bass_kernel_spmd(nc, in_maps, core_ids=list(range(N_CORES)),
                               trace=TRACE)
    LAST_EXEC_NS = res.exec_time_ns
    LAST_RESULT = res
    out = np.concatenate(
        [np.asarray(res.results[c]["out"], dtype=np.float32)
         for c in range(N_CORES)], axis=0)
    return out
